# revision 3
# baseline (speedup 1.0000x reference)
"""Deformable sampling (DCN-style) for Trainium2, 8 cores, batch-parallel.

v5: position-major pipeline around HBM-source dma_gather.

Per core (batch element), positions q in chunks of 1024 (partition =
q%128, row r = q//128). Per (chunk, tap): ONE dma_gather pulls 4096
patch tokens (4 groups x 1024 positions, 512B each: 2x2 corners x 64
group-channels, bf16) from a host-prepped DRAM patch table into
[128, 32, 256] (col = (g*8+r)*256 + e*64 + ch). Weights are computed
position-major ([128, r*144+gk*4+e]) so the multiply broadcasts each
weight over its 64-channel run via a stride-0 AP dim. Corner/tap sums
on DVE; per-group accs DMA to position-major DRAM output; host does
the final [G,HW,Cg] -> [C,H,W] transpose (pure layout).

GPSIMD only generates descriptors (mlp library / InstDMAGatherAnt);
the 16 SDMA engines move all gather bytes.
"""
import sys
import numpy as np
import ml_dtypes

sys.path.insert(0, "/opt/trn_rl_repo")

import concourse.bacc as bacc
import concourse.tile as tile
import concourse.mybir as mybir
from concourse import library_config
from concourse.vector_clock import ScopedClock
from concourse.bass_utils import run_bass_kernel_spmd

F32 = mybir.dt.float32
BF16 = mybir.dt.bfloat16
I16 = mybir.dt.int16

B, C, H, W = 8, 256, 64, 64
G, K, Cg = 4, 9, 64
HW = H * W
KY = np.arange(3).repeat(3)
KX = np.tile(np.arange(3), 3)
NCHUNK = 4
CH = HW // NCHUNK          # 1024 positions per chunk
R = CH // 128              # 8 position-rows per chunk
NTOK = G * HW              # 16384 patch tokens
ES = 256                   # bf16 elems per token (4 corners x 64 ch)
ICOL = K * G * (CH // 16)  # 2304 idx cols per chunk (k, g, scol)
WCOL = R * G * K           # 288 weight cols per chunk (r, g, k)
MAGIC = float(3 << 22)
GSPLIT = 1
SINGLE_PACKET = False
E4 = [(0, 0), (0, 1), (1, 0), (1, 1)]
DELTA = (0, 1, 64, 65)


def _patch_tile_drain():
    if getattr(tile.TileContext, "_drain_patched", False):
        return

    def _drain_and_barrier(self, tick_clock, wait_clock):
        nc = self.nc
        drain_inst = nc.sync.drain()
        wait_clock.add_sem_waits(
            drain_inst.ins, ScopedClock({None: tick_clock.global_clock})
        )
        si = drain_inst.ins.sync_info
        if si is not None and len(si.on_wait) > 1:
            ow = list(si.on_wait)
            si.on_wait = ow[:1]
            for i in range(1, len(ow)):
                nop = nc.sync.nop(nofuse=True, hint="drain_wait_spill")
                nop.ins.sync_info = mybir.SyncInfo(
                    on_wait=[ow[i]], on_update=[]
                )
        nc.all_engine_barrier()
        assert self.sems is not None
        popped = nc._tile_sem_poison_stack.pop()
        assert popped is self._sem_poison
        nc.clear_and_free_semaphores(list(self.sems.allocated().values()))
        nc.all_engine_barrier()

    tile.TileContext._drain_and_barrier = _drain_and_barrier
    tile.TileContext._drain_patched = True


def _floorclamp(nc, out_f, tmp, src, hi):
    """out_f = clamp(floor(src), 0, hi). out_f/tmp scratch, distinct."""
    nc.vector.tensor_scalar(
        out=out_f, in0=src, scalar1=MAGIC, scalar2=MAGIC,
        op0=mybir.AluOpType.add, op1=mybir.AluOpType.subtract,
    )
    nc.vector.tensor_tensor(tmp, out_f, src, mybir.AluOpType.is_gt)
    nc.vector.tensor_tensor(out_f, out_f, tmp, mybir.AluOpType.subtract)
    nc.vector.tensor_scalar(
        out=out_f, in0=out_f, scalar1=0.0, scalar2=hi,
        op0=mybir.AluOpType.max, op1=mybir.AluOpType.min,
    )


def _tent(nc, out, tmp, u):
    """out = relu(1 - |u|)"""
    nc.vector.tensor_scalar_mul(tmp, u, -1.0)
    nc.vector.tensor_tensor(tmp, tmp, u, mybir.AluOpType.max)
    nc.vector.tensor_scalar(
        out=tmp, in0=tmp, scalar1=-1.0, scalar2=1.0,
        op0=mybir.AluOpType.mult, op1=mybir.AluOpType.add,
    )
    nc.vector.tensor_scalar(
        out=out, in0=tmp, scalar1=0.0, scalar2=None,
        op0=mybir.AluOpType.max,
    )


def _build(loop_n=0, mode='full'):
    _patch_tile_drain()
    nc = bacc.Bacc()

    patches = nc.dram_tensor("patches", [NTOK, ES], BF16,
                             kind="ExternalInput")
    ioff = nc.dram_tensor("ioff", [2, 128, NCHUNK * ICOL], F32,
                          kind="ExternalInput")
    igrid = nc.dram_tensor("igrid", [2, 128, NCHUNK * ICOL], F32,
                           kind="ExternalInput")
    goff = nc.dram_tensor("goff", [128, ICOL], F32, kind="ExternalInput")
    woff = nc.dram_tensor("woff", [2, 128, NCHUNK * WCOL], F32,
                          kind="ExternalInput")
    wgrid = nc.dram_tensor("wgrid", [2, 128, NCHUNK * WCOL], F32,
                           kind="ExternalInput")
    maskpm = nc.dram_tensor("maskpm", [128, NCHUNK * WCOL], F32,
                            kind="ExternalInput")
    ypm = nc.dram_tensor("ypm", [G, NCHUNK, 128, R * Cg], F32,
                         kind="ExternalOutput")

    import contextlib

    with tile.TileContext(nc) as tc:
        nc.gpsimd.load_library(library_config.mlp)
        with tc.tile_pool(name="const", bufs=1) as PK, \
             tc.tile_pool(name="itmp", bufs=1) as IT, \
             tc.tile_pool(name="qipool", bufs=2) as PQ, \
             tc.tile_pool(name="wtmp", bufs=1) as WT, \
             tc.tile_pool(name="wqpool", bufs=2) as PW, \
             tc.tile_pool(name="gath", bufs=3) as PG, \
             tc.tile_pool(name="comb", bufs=3) as PC, \
             tc.tile_pool(name="accp", bufs=2) as PA:
            gofft = PK.tile([128, ICOL], F32, tag="goff")
            nc.sync.dma_start(gofft[:], goff[:])
            loop_cm = tc.For_i(0, loop_n, 1) if loop_n else \
                contextlib.nullcontext()
            with loop_cm:
                preps = []
                for ch in range(NCHUNK):
                    preps.append(_prep_chunk(
                        nc, ch, gofft, ioff, igrid, woff, wgrid, maskpm,
                        IT, PQ, WT, PW, mode))
                for ch in range(NCHUNK):
                    _gather_chunk(nc, ch, preps[ch], patches, ypm,
                                  PG, PC, PA, mode)
    nc.finalize()
    return nc


def _prep_chunk(nc, ch, gofft, ioff, igrid, woff, wgrid, maskpm,
                IT, PQ, WT, PW, mode='full'):
    i0, i1 = ch * ICOL, (ch + 1) * ICOL
    w0, w1 = ch * WCOL, (ch + 1) * WCOL
    shi = [128, ICOL]

    # ---- idx path: token index = g*4096 + fy*64 + fx ----
    py = IT.tile(shi, F32, tag="py")
    px = IT.tile(shi, F32, tag="px")
    fa = IT.tile(shi, F32, tag="fa")
    fb = IT.tile(shi, F32, tag="fb")
    nc.sync.dma_start(py[:], igrid[0, :, i0:i1])
    nc.sync.dma_start(px[:], igrid[1, :, i0:i1])
    nc.sync.dma_start(fa[:], ioff[0, :, i0:i1])
    nc.sync.dma_start(fb[:], ioff[1, :, i0:i1])
    nc.vector.tensor_tensor(py[:], py[:], fa[:], mybir.AluOpType.add)
    nc.vector.tensor_tensor(px[:], px[:], fb[:], mybir.AluOpType.add)
    _floorclamp(nc, fa[:], fb[:], py[:], 62.0)
    _floorclamp(nc, fb[:], py[:], px[:], 62.0)
    nc.vector.tensor_scalar_mul(fa[:], fa[:], 64.0)
    nc.vector.tensor_tensor(fa[:], fa[:], fb[:], mybir.AluOpType.add)
    nc.vector.tensor_tensor(fa[:], fa[:], gofft[:], mybir.AluOpType.add)
    qi = PQ.tile(shi, I16, tag=f"qi{ch}", name=f"qi{ch}")
    nc.any.tensor_copy(qi[:], fa[:])
    if mode in ('gather_only', 'idx_only', 'gather_q'):
        if mode == 'gather_q':
            for k in range(K):
                nc.vector.memset(fa[:, k * 256 + 64:(k + 1) * 256], -1.0)
            nc.any.tensor_copy(qi[:], fa[:])
        return (qi, None, fa)

    # ---- weight path (position-major): wq col = r*144 + gk*4 + e ----
    shw = [128, WCOL]
    pw = WT.tile(shw, F32, tag="pw")
    qw = WT.tile(shw, F32, tag="qw")
    ta = WT.tile(shw, F32, tag="ta")
    tb = WT.tile(shw, F32, tag="tb")
    ty0 = WT.tile(shw, F32, tag="ty0")
    ty1 = WT.tile(shw, F32, tag="ty1")
    tx0 = WT.tile(shw, F32, tag="tx0")
    tx1 = WT.tile(shw, F32, tag="tx1")
    msk = WT.tile(shw, F32, tag="msk")
    nc.sync.dma_start(pw[:], wgrid[0, :, w0:w1])
    nc.sync.dma_start(qw[:], wgrid[1, :, w0:w1])
    nc.sync.dma_start(ta[:], woff[0, :, w0:w1])
    nc.sync.dma_start(tb[:], woff[1, :, w0:w1])
    nc.vector.tensor_tensor(pw[:], pw[:], ta[:], mybir.AluOpType.add)
    nc.vector.tensor_tensor(qw[:], qw[:], tb[:], mybir.AluOpType.add)
    nc.sync.dma_start(msk[:], maskpm[:, w0:w1])
    _floorclamp(nc, ta[:], tb[:], pw[:], 62.0)
    nc.vector.tensor_tensor(pw[:], pw[:], ta[:], mybir.AluOpType.subtract)
    _tent(nc, ty0[:], tb[:], pw[:])
    nc.vector.tensor_scalar_add(pw[:], pw[:], -1.0)
    _tent(nc, ty1[:], tb[:], pw[:])
    _floorclamp(nc, ta[:], tb[:], qw[:], 62.0)
    nc.vector.tensor_tensor(qw[:], qw[:], ta[:], mybir.AluOpType.subtract)
    _tent(nc, tx0[:], tb[:], qw[:])
    nc.vector.tensor_scalar_add(qw[:], qw[:], -1.0)
    _tent(nc, tx1[:], tb[:], qw[:])
    wq = PW.tile([128, 4 * WCOL], BF16, tag=f"wq{ch}", name=f"wq{ch}")
    tyl = {0: ty0, 1: ty1}
    txl = {0: tx0, 1: tx1}
    for (ey, ex) in E4:
        e = 2 * ey + ex
        nc.vector.tensor_tensor(ta[:], tyl[ey][:], txl[ex][:],
                                mybir.AluOpType.mult)
        nc.vector.tensor_tensor(
            wq[:, e:e + 4 * (WCOL - 1) + 1:4], ta[:], msk[:],
            mybir.AluOpType.mult)
    return (qi, wq, None)


def _gather_chunk(nc, ch, prep, patches, ypm, PG, PC, PA, mode='full'):
    qi, wq, fa = prep
    if mode in ('gather_only', 'idx_only', 'gather_q'):
        nreg = CH if mode == 'gather_q' else G * CH
        accs = []
        for g in range(G):
            acc_g = PA.tile([128, R * Cg], F32, tag=f"acc{g}",
                            name=f"acc{g}")
            accs.append(acc_g)
        for k in range(K):
            gat = PG.tile([128, G * R * ES], BF16, tag="gat")
            if mode != 'idx_only':
                nc.gpsimd.dma_gather(
                    gat[:].rearrange("p (i e) -> p i e", i=G * R, e=ES),
                    patches[:], qi[:, k * 256:(k + 1) * 256],
                    num_idxs=G * CH, num_idxs_reg=nreg, elem_size=ES,
                    transpose=False, single_packet=False)
            if k == K - 1:
                for g in range(G):
                    src_ap = (fa[:, :R * Cg] if mode == 'idx_only'
                              else gat[:, :R * Cg])
                    nc.any.tensor_copy(accs[g][:], src_ap)
        for g in range(G):
            nc.sync.dma_start(ypm[g, ch], accs[g][:])
        return
    wq4 = wq[:].rearrange("p (r gk e) -> p r gk e", r=R, gk=G * K, e=4)
    accs = []
    for g in range(G):
        acc_g = PA.tile([128, R * Cg], F32, tag=f"acc{g}",
                        name=f"acc{g}")
        accs.append(acc_g)
    for k in range(K):
        gat = PG.tile([128, G * R * ES], BF16, tag="gat")
        nc.gpsimd.dma_gather(
            gat[:].rearrange("p (i e) -> p i e", i=G * R, e=ES),
            patches[:], qi[:, k * 256:(k + 1) * 256],
            num_idxs=G * CH, num_idxs_reg=G * CH, elem_size=ES,
            transpose=False, single_packet=False)
        for g in range(G):
            gsl = gat[:, g * R * ES:(g + 1) * R * ES].rearrange(
                "p (r e c) -> p r e c", r=R, e=4, c=Cg)
            wb = wq4[:, :, g * K + k, :].unsqueeze(3).broadcast_to(
                (128, R, 4, Cg))
            prod = PC.tile([128, R * 4 * Cg], BF16, tag="prod")
            prod4 = prod[:].rearrange("p (r e c) -> p r e c",
                                      r=R, e=4, c=Cg)
            nc.vector.tensor_tensor(prod4, gsl, wb, mybir.AluOpType.mult)
            s0 = PC.tile([128, R * Cg], BF16, tag="s0")
            s1 = PC.tile([128, R * Cg], BF16, tag="s1")
            s03 = s0[:].rearrange("p (r c) -> p r c", r=R, c=Cg)
            s13 = s1[:].rearrange("p (r c) -> p r c", r=R, c=Cg)
            nc.vector.tensor_tensor(s03, prod4[:, :, 0, :],
                                    prod4[:, :, 1, :], mybir.AluOpType.add)
            nc.vector.tensor_tensor(s13, prod4[:, :, 2, :],
                                    prod4[:, :, 3, :], mybir.AluOpType.add)
            nc.vector.tensor_tensor(s03, s03, s13, mybir.AluOpType.add)
            if k == 0:
                nc.any.tensor_copy(accs[g][:], s0[:])
            else:
                nc.vector.tensor_tensor(accs[g][:], accs[g][:], s0[:],
                                        mybir.AluOpType.add)
    for g in range(G):
        nc.sync.dma_start(ypm[g, ch], accs[g][:])


def _host_prep(input_b, offset_b, mask_b, consts):
    X = np.asarray(input_b, dtype=np.float32).reshape(C, HW)
    off = np.asarray(offset_b, dtype=np.float32).reshape(G, K, 2, HW)
    msk = np.asarray(mask_b, dtype=np.float32).reshape(G, K, HW)

    # patches [g*4096 + t, e*64 + ch] = X[g*64+ch, t + DELTA[e]]
    sh = np.zeros((4, C, HW), dtype=np.float32)
    for e, d in enumerate(DELTA):
        sh[e, :, :HW - d] = X[:, d:]
    # [e, g, ch, t] -> [g, t, e, ch]
    patches = np.ascontiguousarray(
        sh.reshape(4, G, Cg, HW).transpose(1, 3, 0, 2).reshape(NTOK, ES)
    ).astype(ml_dtypes.bfloat16)

    # idx offsets: [c, p, ch*ICOL + k*256 + g*64 + scol] = off[g,k,c,q]
    # q = ch*1024 + scol*16 + lane,  p = rep*16 + lane
    d = off.transpose(2, 0, 1, 3).reshape(2, G, K, NCHUNK, 64, 16)
    d = d.transpose(0, 5, 3, 2, 1, 4)       # [c, lane, ch, k, g, scol]
    d = np.broadcast_to(d[:, None], (2, 8, 16, NCHUNK, K, G, 64))
    d = d.transpose(0, 3, 4, 5, 6, 1, 2)    # [c, ch, k, g, scol, rep, lane]
    # want [c, p=rep*16+lane, ch, k, g, scol]:
    d2 = off.transpose(2, 0, 1, 3).reshape(2, G, K, NCHUNK, 64, 16)
    d2 = d2.transpose(0, 3, 2, 1, 4, 5)     # [c, ch, k, g, scol, lane]
    d2 = np.broadcast_to(d2[:, :, :, :, :, None, :],
                         (2, NCHUNK, K, G, 64, 8, 16))
    ioff = np.ascontiguousarray(
        d2.transpose(0, 5, 6, 1, 2, 3, 4).reshape(2, 128, NCHUNK * ICOL))

    # weight offsets (position-major): [c, p, ch*WCOL + r*36 + g*9 + k]
    # = off[g,k,c, ch*1024 + r*128 + p]
    dw = off.transpose(2, 0, 1, 3).reshape(2, G, K, NCHUNK, R, 128)
    woff = np.ascontiguousarray(
        dw.transpose(0, 5, 3, 4, 1, 2).reshape(2, 128, NCHUNK * WCOL))
    mw = msk.reshape(G, K, NCHUNK, R, 128)
    maskpm = np.ascontiguousarray(
        mw.transpose(4, 2, 3, 0, 1).reshape(128, NCHUNK * WCOL))

    return {
        "patches": patches,
        "ioff": ioff,
        "igrid": consts["igrid"],
        "goff": consts["goff"],
        "woff": woff,
        "wgrid": consts["wgrid"],
        "maskpm": maskpm,
    }


def _consts():
    lane = np.arange(16)
    scol = np.arange(64)
    chv = np.arange(NCHUNK)
    # igrid[c, p, ch*ICOL + k*256 + g*64 + scol]:
    # q = ch*1024 + scol*16 + lane
    q = (chv[:, None, None] * 1024 + scol[None, :, None] * 16
         + lane[None, None, :])                      # [ch, scol, lane]
    igrid = np.empty((2, 128, NCHUNK * ICOL), dtype=np.float32)
    for c in range(2):
        kk = (KY if c == 0 else KX)
        base = (q[None] // 64 if c == 0 else q[None] % 64) - 1  # [1,ch,s,l]
        v = base + kk[:, None, None, None]           # [k, ch, scol, lane]
        v = np.broadcast_to(v[:, :, :, None, :, None],
                            (K, NCHUNK, 64, G, 16, 8))
        # [ch, k, g, scol] cols, partition rep*16+lane
        v2 = base + kk[:, None, None, None]          # [k, ch, scol, lane]
        v2 = np.broadcast_to(v2[None, :, :, :, :],
                             (G, K, NCHUNK, 64, 16))  # [g,k,ch,scol,lane]
        v2 = np.broadcast_to(v2[:, :, :, :, None, :],
                             (G, K, NCHUNK, 64, 8, 16))
        igrid[c] = v2.transpose(4, 5, 2, 1, 0, 3).reshape(128,
                                                          NCHUNK * ICOL)
    goff = np.broadcast_to(
        (np.arange(G) * HW).astype(np.float32)[None, None, :, None],
        (128, K, G, 64)).reshape(128, ICOL).copy()

    # wgrid[c, p, ch*WCOL + r*36 + g*9 + k]: q = ch*1024 + r*128 + p
    p = np.arange(128)
    qw = (chv[:, None, None] * 1024 + np.arange(R)[None, :, None] * 128
          + p[None, None, :])                        # [ch, r, p]
    wgrid = np.empty((2, 128, NCHUNK * WCOL), dtype=np.float32)
    for c in range(2):
        kk = (KY if c == 0 else KX)
        base = (qw // 64 if c == 0 else qw % 64) - 1  # [ch, r, p]
        v = base[None, None] + kk[None, :, None, None, None]  # [1,k,ch,r,p]
        v = np.broadcast_to(v, (G, K, NCHUNK, R, 128))
        wgrid[c] = v.transpose(4, 2, 3, 0, 1).reshape(128, NCHUNK * WCOL)
    return {"igrid": igrid, "goff": goff, "wgrid": wgrid}


_STATE = {}


def kernel(input, offset, mask):
    if "nc" not in _STATE:
        _STATE["nc"] = _build()
        _STATE["consts"] = _consts()
    nc = _STATE["nc"]
    consts = _STATE["consts"]
    in_maps = [
        _host_prep(np.asarray(input[b]), np.asarray(offset[b]),
                   np.asarray(mask[b]), consts)
        for b in range(B)
    ]
    res = run_bass_kernel_spmd(nc, in_maps, core_ids=list(range(B)))
    out = np.empty((B, C, H, W), dtype=np.float32)
    for b in range(B):
        ypm = np.asarray(res.results[b]["ypm"]).reshape(G, NCHUNK, 128,
                                                        R, Cg)
        # out[g*64+c, ch*1024 + r*128 + p] = ypm[g, ch, p, r, c]
        full = ypm.transpose(0, 4, 1, 3, 2).reshape(C, HW)
        out[b] = full.reshape(C, H, W)
    return out
